# revision 12
# baseline (speedup 1.0000x reference)
"""AWQ 4-bit quantized linear (x @ dequant(qweight).T + bias) on 8 Trainium2 cores.

Column-parallel sharding: out_features (O=11008) split exactly across 8 cores
(O_sh=1376); x replicated.  v2 design: the packed qweight is transposed on the
HOST (pure layout move, like xT) into [128, 4, O_sh] int32 so the device-side
unpack lands nibbles directly on matmul k-tile partitions — no PE transposes
and no xbar weight transposes.  x rows are host-permuted to match the nibble
k-tile order: k-tile kt=(nib*4+c) covers original input rows
i = 8*(c*128+p)+nib for p in [0,128).  Quant groups (GS=128) stay aligned:
g = 8c + p//16 for every nibble, so per-group z/s become per-(partition,c)
broadcast tensors along the o free dim.

Dequant is ~100 large DVE ops total: unpack (i32), subtract z_bc (-> f16 into
resident WT), in-place multiply by s_bc.  Phase B1 warms the PE o-slice-major
over 3 resident x t-chunks while dequant streams; its epilogues run on
ACT(psum copy) + GpSimd(bias add) so the DVE FIFO stays clear for dequant.
Phase B2 is the baseline t-outer streaming loop (512/512/352 n-splits).

  kernel(x, qweight, qzeros, scales, bias) -> [8192, 11008] fp16
"""

import numpy as np
from contextlib import ExitStack

import concourse.bacc as bacc
import concourse.mybir as mybir
import concourse.tile as tile
from concourse._compat import with_exitstack
from concourse.bass_utils import run_bass_kernel_spmd


class _Bacc(bacc.Bacc):
    """Bacc that keeps matmuls self-loading.

    The stock `move_matmul_waits_to_ldweights` pass splits every InstMatmult
    into an explicit InstLdweights + InstMatmult; explicit LDWEIGHTS skips
    walrus's fast-weight-load codegen and measured ~117ns per matmul (~45ns
    un-hidden PE stall each). Self-loading matmuls let walrus emit the
    optimized weight load.
    """

    def move_matmul_waits_to_ldweights(self):
        pass


PACK = 8      # int32 packs 8 x 4-bit values, low nibble first
QBIT = 4
GS = 128      # quant group size == matmul k-tile size
NCORES = 8
TCH = 256     # t-columns fetched per x-tile DMA (2 PSUM t-tiles)
NB1 = 3       # t-chunks processed o-slice-major during the dequant window

f16 = mybir.dt.float16
i32 = mybir.dt.int32
f32 = mybir.dt.float32
LSR = mybir.AluOpType.logical_shift_right
AND = mybir.AluOpType.bitwise_and
SUB = mybir.AluOpType.subtract
MUL = mybir.AluOpType.mult
ADD = mybir.AluOpType.add
IDENT = mybir.ActivationFunctionType.Identity


def _n_splits(o_sh):
    # largest-first 512,512,352 pattern: 512-wide matmuls amortize issue
    splits, off = [], 0
    while off < o_sh:
        n = min(512, o_sh - off)
        splits.append((off, n))
        off += n
    return splits


def _os_slices(o_sh, n):
    # n roughly-equal o-slices, 16-aligned
    base = (o_sh // n) & ~15
    offs, out = 0, []
    for s in range(n):
        w = base if s < n - 1 else o_sh - offs
        out.append((offs, w))
        offs += w
    return out


@with_exitstack
def _emit(ctx, tc, T, I, O_SH, xT, qwT, qz, scT_d, zT_dram, b, out):
    nc = tc.nc
    KT = I // 128          # k-tiles (== 32)
    NG = I // GS           # quant groups (== 32)
    NC4 = KT // PACK       # qwT c-chunks (== 4)
    OT = -(-O_SH // 128)   # 128-row o-tiles (z-prep layout)
    assert I % (128 * PACK) == 0 and T % TCH == 0 and O_SH % 16 == 0

    const_pool = ctx.enter_context(tc.tile_pool(name="const", bufs=1))
    wt_pool = ctx.enter_context(tc.tile_pool(name="wt", bufs=1))
    deq_pool = ctx.enter_context(tc.tile_pool(name="deq", bufs=1))
    x_pool = ctx.enter_context(tc.tile_pool(name="x", bufs=3))
    o_pool = ctx.enter_context(tc.tile_pool(name="o", bufs=2))
    ps1_pool = ctx.enter_context(tc.tile_pool(name="ps1", bufs=2, space="PSUM"))
    ps_pool = ctx.enter_context(tc.tile_pool(name="ps", bufs=2, space="PSUM"))

    OSL = _os_slices(O_SH, 4)

    # ---- constants / prep (ACT dma queue; DVE compute) ----
    bias_bc = const_pool.tile([128, O_SH], f16)
    nc.scalar.dma_start(bias_bc[:], b.broadcast_to([128, O_SH]))

    # s_bc[p, c, o] = scales[o, 8c + p//16]  (broadcast from host-transposed scT)
    s_bc = const_pool.tile([128, NC4, O_SH], f16)
    for c in range(NC4):
        for m in range(PACK):
            g = c * PACK + m
            nc.scalar.dma_start(
                s_bc[m * 16 : (m + 1) * 16, c, :],
                scT_d[g : g + 1, :].broadcast_to([16, O_SH]),
            )

    # z unpack in [o-part, g] layout, per-j xbar transpose (128-col padded),
    # bounce through DRAM, broadcast to z_bc
    zq = const_pool.tile([128, OT, NG // PACK], i32)
    for j in range(OT):
        rj = min(128, O_SH - j * 128)
        nc.scalar.dma_start(zq[:rj, j, :], qz[j * 128 : j * 128 + rj, :])
    zi = const_pool.tile([128, OT, NG], i32)
    for m in range(PACK):
        nc.vector.tensor_scalar(
            zi.rearrange("p o (c m) -> p o m c", m=PACK)[:, :, m, :],
            zq[:], QBIT * m, 0xF, LSR, AND,
        )
    zf_pad = const_pool.tile([128, OT, 128], f16)
    nc.gpsimd.memset(zf_pad[:], 0.0)
    nc.vector.tensor_copy(zf_pad[:, :, :NG], zi[:])
    zT_pad = const_pool.tile([128, O_SH], f16)
    for j in range(OT):
        rj = min(128, O_SH - j * 128)
        nc.scalar.dma_start_transpose(
            zT_pad[:, j * 128 : j * 128 + rj], zf_pad[:rj, j, :]
        )
    nc.scalar.dma_start(zT_dram[:], zT_pad[:NG, :])
    z_bc = const_pool.tile([128, NC4, O_SH], f16)
    for c in range(NC4):
        for m in range(PACK):
            g = c * PACK + m
            nc.scalar.dma_start(
                z_bc[m * 16 : (m + 1) * 16, c, :],
                zT_dram[g : g + 1, :].broadcast_to([16, O_SH]),
            )

    # Resident dequantized weights: [128 (p), KT, O_SH] fp16
    WT = wt_pool.tile([128, KT, O_SH], f16)

    # qwT staged in SBUF whole (o-sliced DMAs interleave with early x tiles
    # on the sync queue so neither stream starves the other)
    qw_sb = const_pool.tile([128, NC4, O_SH], i32)

    def dequant_slice(si):
        noff, nsz = OSL[si]
        osl = slice(noff, noff + nsz)
        for nib in range(PACK):
            kt4 = slice(nib * NC4, (nib + 1) * NC4)
            u = deq_pool.tile([128, NC4, 512], i32, tag="u", name="u")
            nc.vector.tensor_scalar(
                u[:, :, :nsz], qw_sb[:, :, osl], QBIT * nib, 0xF, LSR, AND
            )
            # v = u - z  (cast to f16 on write, into WT directly)
            nc.vector.tensor_tensor(
                WT[:, kt4, osl], u[:, :, :nsz], z_bc[:, :, osl], SUB
            )
            # WT *= s  (in-place f16)
            nc.vector.tensor_tensor(
                WT[:, kt4, osl], WT[:, kt4, osl], s_bc[:, :, osl], MUL
            )

    xT_r = xT.rearrange("(kt p) t -> p kt t", p=128)  # [128, KT, T]

    def load_x(ti):
        xt = x_pool.tile([128, KT, TCH], f16, tag="xt", name="xt")
        nc.sync.dma_start(xt[:], xT_r[:, :, ti * TCH : (ti + 1) * TCH])
        return xt

    # sync-queue interleave: qwT slices + first B1 x tiles
    nc.sync.dma_start(qw_sb[:, :, OSL[0][0] : OSL[0][0] + OSL[0][1]],
                      qwT[:, :, OSL[0][0] : OSL[0][0] + OSL[0][1]])
    nc.sync.dma_start(qw_sb[:, :, OSL[1][0] : OSL[1][0] + OSL[1][1]],
                      qwT[:, :, OSL[1][0] : OSL[1][0] + OSL[1][1]])
    b1_tiles = [load_x(0)]
    nc.sync.dma_start(qw_sb[:, :, OSL[2][0] : OSL[2][0] + OSL[2][1]],
                      qwT[:, :, OSL[2][0] : OSL[2][0] + OSL[2][1]])
    nc.sync.dma_start(qw_sb[:, :, OSL[3][0] : OSL[3][0] + OSL[3][1]],
                      qwT[:, :, OSL[3][0] : OSL[3][0] + OSL[3][1]])
    nb1 = min(NB1, T // TCH)
    for ti in range(1, nb1):
        b1_tiles.append(load_x(ti))

    # ---- phase B1: o-slice-major over the resident t-chunks ----
    def b1_chain(xt, ti, h, noff, nsz):
        tsl = slice(h * 128, (h + 1) * 128)
        ps = ps1_pool.tile([128, nsz], f32, tag="psb1", name="psb1",
                           padded_shape=[128, 512])
        for kt in range(KT):
            nc.tensor.matmul(
                ps[:], xt[:, kt, tsl], WT[:, kt, noff : noff + nsz],
                start=(kt == 0), stop=(kt == KT - 1),
            )
        ot = o_pool.tile([128, nsz], f16, tag="otb1", name="otb1",
                         padded_shape=[128, 512])
        nc.scalar.activation(ot[:], ps[:], IDENT)
        nc.gpsimd.tensor_tensor(ot[:], ot[:], bias_bc[:, noff : noff + nsz], ADD)
        t0 = ti * TCH + h * 128
        nc.scalar.dma_start(out[t0 : t0 + 128, noff : noff + nsz], ot[:])

    for si in range(len(OSL)):
        dequant_slice(si)
        noff, nsz = OSL[si]
        for ti in range(nb1):
            for h in range(TCH // 128):
                b1_chain(b1_tiles[ti], ti, h, noff, nsz)

    # ---- phase B2: stream remaining t through the PE (baseline structure) ----
    splits = _n_splits(O_SH)

    def chains(ti):
        xt = load_x(ti)
        for h in range(TCH // 128):
            tsl = slice(h * 128, (h + 1) * 128)
            psums = [
                ps_pool.tile([128, nsz], f32, tag=f"ps{noff}", name=f"ps{noff}")
                for noff, nsz in splits
            ]
            for k in range(KT):
                for ps, (noff, nsz) in zip(psums, splits):
                    nc.tensor.matmul(
                        ps[:],
                        xt[:, k, tsl],
                        WT[:, k, noff : noff + nsz],
                        start=(k == 0),
                        stop=(k == KT - 1),
                    )
            t0 = ti * TCH + h * 128
            for ps, (noff, nsz) in zip(psums, splits):
                ot = o_pool.tile([128, nsz], f16, tag=f"ot{noff}", name=f"ot{noff}")
                nc.vector.tensor_tensor(
                    ot[:], ps[:], bias_bc[:, noff : noff + nsz], ADD
                )
                nc.scalar.dma_start(out[t0 : t0 + 128, noff : noff + nsz], ot[:])

    for ti in range(nb1, T // TCH):
        chains(ti)


def _build(T, I, O_SH):
    nc = _Bacc(
        "TRN2",
        target_bir_lowering=False,
        debug=False,
        enable_asserts=False,
        num_devices=NCORES,
    )
    NC4 = I // 128 // PACK
    NG = I // GS
    xT_d = nc.dram_tensor("xT", [I, T], f16, kind="ExternalInput")
    qwT_d = nc.dram_tensor("qwT", [128, NC4, O_SH], i32, kind="ExternalInput")
    qz_d = nc.dram_tensor("qz", [O_SH, NG // PACK], i32, kind="ExternalInput")
    scT_d = nc.dram_tensor("scT", [NG, O_SH], f16, kind="ExternalInput")
    zT_dram = nc.dram_tensor("zT_scratch", [NG, O_SH], f16, kind="Internal")
    b_d = nc.dram_tensor("b", [1, O_SH], f16, kind="ExternalInput")
    out_d = nc.dram_tensor("out", [T, O_SH], f16, kind="ExternalOutput")
    with tile.TileContext(nc) as tc:
        _emit(
            tc, T, I, O_SH,
            xT_d.ap(), qwT_d.ap(), qz_d.ap(), scT_d.ap(), zT_dram.ap(),
            b_d.ap(), out_d.ap(),
        )
    nc.compile()
    return nc


_NC_CACHE = {}


def _get_nc(T, I, O_SH):
    key = (T, I, O_SH)
    if key not in _NC_CACHE:
        _NC_CACHE[key] = _build(*key)
    return _NC_CACHE[key]


def _shard_inputs(x, qweight, qzeros, scales, bias):
    T, I = x.shape
    O = qweight.shape[0]
    o_pad = -(-O // (16 * NCORES)) * (16 * NCORES)
    o_sh = o_pad // NCORES
    KT = I // 128

    # x rows permuted to nibble k-tile order: row kt*128+p <- i=8*((kt%4)*128+p)+kt//4
    kt = np.arange(KT)
    p = np.arange(128)
    idx = (8 * ((kt % 4)[:, None] * 128 + p[None, :]) + (kt // 4)[:, None]).reshape(-1)
    xT = np.ascontiguousarray(np.asarray(x).T[idx])

    def pad_rows(a):
        if a.shape[0] == o_pad:
            return a
        pad = np.zeros((o_pad - a.shape[0],) + a.shape[1:], a.dtype)
        return np.concatenate([a, pad], axis=0)

    qw_p = pad_rows(np.asarray(qweight))
    qz_p = pad_rows(np.asarray(qzeros))
    sc_p = pad_rows(np.asarray(scales))
    b_p = pad_rows(np.asarray(bias))
    in_maps = []
    for c in range(NCORES):
        rows = slice(c * o_sh, (c + 1) * o_sh)
        # packed-weight transpose (layout only): qwT[p, c4, o] = qw[o, c4*128+p]
        qwT = np.ascontiguousarray(
            qw_p[rows].T.reshape(4, 128, o_sh).transpose(1, 0, 2)
        )
        scs = np.ascontiguousarray(sc_p[rows])
        in_maps.append(
            {
                "xT": xT,
                "qwT": qwT,
                "qz": np.ascontiguousarray(qz_p[rows]),
                "scT": np.ascontiguousarray(scs.T),
                "b": np.ascontiguousarray(b_p[rows]).reshape(1, o_sh),
            }
        )
    return in_maps, T, I, O, o_sh


def _run(x, qweight, qzeros, scales, bias, trace=False, **kw):
    in_maps, T, I, O, o_sh = _shard_inputs(x, qweight, qzeros, scales, bias)
    nc = _get_nc(T, I, o_sh)
    res = run_bass_kernel_spmd(nc, in_maps, list(range(NCORES)), trace=trace, **kw)
    out = np.concatenate([res.results[c]["out"] for c in range(NCORES)], axis=1)
    return out[:, :O], res


def kernel(x, qweight, qzeros, scales, bias):
    out, _ = _run(x, qweight, qzeros, scales, bias)
    return out


# revision 20
# speedup vs baseline: 1.0126x; 1.0126x over previous
"""AWQ 4-bit quantized linear (x @ dequant(qweight).T + bias) on 8 Trainium2 cores.

Column-parallel sharding: out_features (O=11008) split exactly across 8 cores
(O_sh=1376); x replicated.  v2 design: the packed qweight is transposed on the
HOST (pure layout move, like xT) into [128, 4, O_sh] int32 so the device-side
unpack lands nibbles directly on matmul k-tile partitions — no PE transposes
and no xbar weight transposes.  x rows are host-permuted to match the nibble
k-tile order: k-tile kt=(nib*4+c) covers original input rows
i = 8*(c*128+p)+nib for p in [0,128).  Quant groups (GS=128) stay aligned:
g = 8c + p//16 for every nibble, so per-group z/s become per-(partition,c)
broadcast tensors along the o free dim.

Dequant is ~100 large DVE ops total: unpack (i32), subtract z_bc (-> f16 into
resident WT), in-place multiply by s_bc.  Phase B1 warms the PE o-slice-major
over 3 resident x t-chunks while dequant streams; its epilogues run on
ACT(psum copy) + GpSimd(bias add) so the DVE FIFO stays clear for dequant.
Phase B2 is the baseline t-outer streaming loop (512/512/352 n-splits).

  kernel(x, qweight, qzeros, scales, bias) -> [8192, 11008] fp16
"""

import numpy as np
from contextlib import ExitStack

import concourse.bacc as bacc
import concourse.mybir as mybir
import concourse.tile as tile
from concourse._compat import with_exitstack
from concourse.bass_utils import run_bass_kernel_spmd


class _Bacc(bacc.Bacc):
    """Bacc that keeps matmuls self-loading.

    The stock `move_matmul_waits_to_ldweights` pass splits every InstMatmult
    into an explicit InstLdweights + InstMatmult; explicit LDWEIGHTS skips
    walrus's fast-weight-load codegen and measured ~117ns per matmul (~45ns
    un-hidden PE stall each). Self-loading matmuls let walrus emit the
    optimized weight load.
    """

    def move_matmul_waits_to_ldweights(self):
        pass


PACK = 8      # int32 packs 8 x 4-bit values, low nibble first
QBIT = 4
GS = 128      # quant group size == matmul k-tile size
NCORES = 8
TCH = 256     # t-columns fetched per x-tile DMA (2 PSUM t-tiles)
NB1 = 3       # t-chunks processed o-slice-major during the dequant window

f16 = mybir.dt.float16
i32 = mybir.dt.int32
f32 = mybir.dt.float32
LSR = mybir.AluOpType.logical_shift_right
AND = mybir.AluOpType.bitwise_and
SUB = mybir.AluOpType.subtract
MUL = mybir.AluOpType.mult
ADD = mybir.AluOpType.add
IDENT = mybir.ActivationFunctionType.Identity


def _n_splits(o_sh):
    # largest-first 512,512,352 pattern: 512-wide matmuls amortize issue
    splits, off = [], 0
    while off < o_sh:
        n = min(512, o_sh - off)
        splits.append((off, n))
        off += n
    return splits


def _os_slices(o_sh, n):
    # n roughly-equal o-slices, 16-aligned
    base = (o_sh // n) & ~15
    offs, out = 0, []
    for s in range(n):
        w = base if s < n - 1 else o_sh - offs
        out.append((offs, w))
        offs += w
    return out


@with_exitstack
def _emit(ctx, tc, T, I, O_SH, xT, qwT, qz, scT_d, zT_dram, b, out):
    nc = tc.nc
    KT = I // 128          # k-tiles (== 32)
    NG = I // GS           # quant groups (== 32)
    NC4 = KT // PACK       # qwT c-chunks (== 4)
    OT = -(-O_SH // 128)   # 128-row o-tiles (z-prep layout)
    assert I % (128 * PACK) == 0 and T % TCH == 0 and O_SH % 16 == 0

    const_pool = ctx.enter_context(tc.tile_pool(name="const", bufs=1))
    wt_pool = ctx.enter_context(tc.tile_pool(name="wt", bufs=1))
    deq_pool = ctx.enter_context(tc.tile_pool(name="deq", bufs=1))
    x_pool = ctx.enter_context(tc.tile_pool(name="x", bufs=3))
    o_pool = ctx.enter_context(tc.tile_pool(name="o", bufs=2))
    ps1_pool = ctx.enter_context(tc.tile_pool(name="ps1", bufs=2, space="PSUM"))
    ps_pool = ctx.enter_context(tc.tile_pool(name="ps", bufs=2, space="PSUM"))

    OSL = _os_slices(O_SH, 4)

    # ---- constants / prep (ACT dma queue; DVE compute) ----
    bias_bc = const_pool.tile([128, O_SH], f16)
    nc.scalar.dma_start(bias_bc[:], b.broadcast_to([128, O_SH]))

    # s_bc[p, c, o] = scales[o, 8c + p//16]: 8 broadcast DMAs (one per nibble
    # row m), each expanding [1, 4, O_SH] -> 16 partitions
    scT_m = scT_d.rearrange("(c m) o -> m c o", m=PACK)  # [8, 4, O_SH]
    s_bc = const_pool.tile([128, NC4, O_SH], f16)
    for m in range(PACK):
        nc.scalar.dma_start(
            s_bc[m * 16 : (m + 1) * 16, :, :],
            scT_m[m : m + 1].broadcast_to([16, NC4, O_SH]),
        )

    # z unpack in [o-part, g] layout, per-j xbar transpose (128-col padded),
    # then the same 16-way partition expansion
    zq = const_pool.tile([128, OT, NG // PACK], i32)
    jf = O_SH // 128  # full o-tiles
    nc.scalar.dma_start(
        zq[:, :jf, :], qz[: jf * 128, :].rearrange("(j p) g -> p j g", p=128)
    )
    if O_SH % 128:
        nc.scalar.dma_start(zq[: O_SH % 128, jf, :], qz[jf * 128 :, :])
    zi = const_pool.tile([128, OT, NG], i32)
    for m in range(PACK):
        nc.vector.tensor_scalar(
            zi.rearrange("p o (c m) -> p o m c", m=PACK)[:, :, m, :],
            zq[:], QBIT * m, 0xF, LSR, AND,
        )
    zf_pad = const_pool.tile([128, OT, 128], f16)
    nc.gpsimd.memset(zf_pad[:], 0.0)
    nc.vector.tensor_copy(zf_pad[:, :, :NG], zi[:])
    zT_pad = const_pool.tile([128, O_SH], f16)
    for j in range(OT):
        rj = min(128, O_SH - j * 128)
        nc.scalar.dma_start_transpose(
            zT_pad[:, j * 128 : j * 128 + rj], zf_pad[:rj, j, :]
        )
    nc.scalar.dma_start(zT_dram[:], zT_pad[:NG, :])
    zT_m = zT_dram.rearrange("(c m) o -> m c o", m=PACK)
    z_bc = const_pool.tile([128, NC4, O_SH], f16)
    for m in range(PACK):
        nc.scalar.dma_start(
            z_bc[m * 16 : (m + 1) * 16, :, :],
            zT_m[m : m + 1].broadcast_to([16, NC4, O_SH]),
        )

    # Resident dequantized weights: [128 (p), KT, O_SH] fp16
    WT = wt_pool.tile([128, KT, O_SH], f16)

    # qwT staged in SBUF whole (o-sliced DMAs interleave with early x tiles
    # on the sync queue so neither stream starves the other)
    qw_sb = const_pool.tile([128, NC4, O_SH], i32)

    def dequant_slice(si):
        noff, nsz = OSL[si]
        osl = slice(noff, noff + nsz)
        for nib in range(PACK):
            kt4 = slice(nib * NC4, (nib + 1) * NC4)
            u = deq_pool.tile([128, NC4, 512], i32, tag="u", name="u")
            nc.vector.tensor_scalar(
                u[:, :, :nsz], qw_sb[:, :, osl], QBIT * nib, 0xF, LSR, AND
            )
            # v = u - z  (cast to f16 on write, into WT directly)
            nc.vector.tensor_tensor(
                WT[:, kt4, osl], u[:, :, :nsz], z_bc[:, :, osl], SUB
            )
            # WT *= s  (in-place f16)
            nc.vector.tensor_tensor(
                WT[:, kt4, osl], WT[:, kt4, osl], s_bc[:, :, osl], MUL
            )

    xT_r = xT.rearrange("(kt p) t -> p kt t", p=128)  # [128, KT, T]

    def load_x(ti):
        xt = x_pool.tile([128, KT, TCH], f16, tag="xt", name="xt")
        nc.sync.dma_start(xt[:], xT_r[:, :, ti * TCH : (ti + 1) * TCH])
        return xt

    # sync-queue interleave: qwT slices + first B1 x tiles
    nc.sync.dma_start(qw_sb[:, :, OSL[0][0] : OSL[0][0] + OSL[0][1]],
                      qwT[:, :, OSL[0][0] : OSL[0][0] + OSL[0][1]])
    nc.sync.dma_start(qw_sb[:, :, OSL[1][0] : OSL[1][0] + OSL[1][1]],
                      qwT[:, :, OSL[1][0] : OSL[1][0] + OSL[1][1]])
    b1_tiles = [load_x(0)]
    nc.sync.dma_start(qw_sb[:, :, OSL[2][0] : OSL[2][0] + OSL[2][1]],
                      qwT[:, :, OSL[2][0] : OSL[2][0] + OSL[2][1]])
    nc.sync.dma_start(qw_sb[:, :, OSL[3][0] : OSL[3][0] + OSL[3][1]],
                      qwT[:, :, OSL[3][0] : OSL[3][0] + OSL[3][1]])
    nb1 = min(NB1, T // TCH)
    for ti in range(1, nb1):
        b1_tiles.append(load_x(ti))

    # ---- phase B1: o-slice-major over the resident t-chunks ----
    def b1_chain(xt, ti, h, noff, nsz):
        tsl = slice(h * 128, (h + 1) * 128)
        ps = ps1_pool.tile([128, nsz], f32, tag="psb1", name="psb1",
                           padded_shape=[128, 512])
        for kt in range(KT):
            nc.tensor.matmul(
                ps[:], xt[:, kt, tsl], WT[:, kt, noff : noff + nsz],
                start=(kt == 0), stop=(kt == KT - 1),
            )
        ot = o_pool.tile([128, nsz], f16, tag="otb1", name="otb1",
                         padded_shape=[128, 512])
        nc.scalar.activation(ot[:], ps[:], IDENT)
        nc.gpsimd.tensor_tensor(ot[:], ot[:], bias_bc[:, noff : noff + nsz], ADD)
        t0 = ti * TCH + h * 128
        nc.scalar.dma_start(out[t0 : t0 + 128, noff : noff + nsz], ot[:])

    for si in range(len(OSL)):
        dequant_slice(si)
        noff, nsz = OSL[si]
        for ti in range(nb1):
            for h in range(TCH // 128):
                b1_chain(b1_tiles[ti], ti, h, noff, nsz)

    # ---- phase B2: stream remaining t through the PE (baseline structure) ----
    splits = _n_splits(O_SH)

    def chains(ti):
        xt = load_x(ti)
        for h in range(TCH // 128):
            tsl = slice(h * 128, (h + 1) * 128)
            psums = [
                ps_pool.tile([128, nsz], f32, tag=f"ps{noff}", name=f"ps{noff}")
                for noff, nsz in splits
            ]
            for k in range(KT):
                for ps, (noff, nsz) in zip(psums, splits):
                    nc.tensor.matmul(
                        ps[:],
                        xt[:, k, tsl],
                        WT[:, k, noff : noff + nsz],
                        start=(k == 0),
                        stop=(k == KT - 1),
                    )
            t0 = ti * TCH + h * 128
            for ps, (noff, nsz) in zip(psums, splits):
                ot = o_pool.tile([128, nsz], f16, tag=f"ot{noff}", name=f"ot{noff}")
                nc.vector.tensor_tensor(
                    ot[:], ps[:], bias_bc[:, noff : noff + nsz], ADD
                )
                nc.scalar.dma_start(out[t0 : t0 + 128, noff : noff + nsz], ot[:])

    for ti in range(nb1, T // TCH):
        chains(ti)


def _build(T, I, O_SH):
    nc = _Bacc(
        "TRN2",
        target_bir_lowering=False,
        debug=False,
        enable_asserts=False,
        num_devices=NCORES,
    )
    NC4 = I // 128 // PACK
    NG = I // GS
    xT_d = nc.dram_tensor("xT", [I, T], f16, kind="ExternalInput")
    qwT_d = nc.dram_tensor("qwT", [128, NC4, O_SH], i32, kind="ExternalInput")
    qz_d = nc.dram_tensor("qz", [O_SH, NG // PACK], i32, kind="ExternalInput")
    scT_d = nc.dram_tensor("scT", [NG, O_SH], f16, kind="ExternalInput")
    zT_dram = nc.dram_tensor("zT_scratch", [NG, O_SH], f16, kind="Internal")
    b_d = nc.dram_tensor("b", [1, O_SH], f16, kind="ExternalInput")
    out_d = nc.dram_tensor("out", [T, O_SH], f16, kind="ExternalOutput")
    with tile.TileContext(nc) as tc:
        _emit(
            tc, T, I, O_SH,
            xT_d.ap(), qwT_d.ap(), qz_d.ap(), scT_d.ap(), zT_dram.ap(),
            b_d.ap(), out_d.ap(),
        )
    nc.compile()
    return nc


_NC_CACHE = {}


def _get_nc(T, I, O_SH):
    key = (T, I, O_SH)
    if key not in _NC_CACHE:
        _NC_CACHE[key] = _build(*key)
    return _NC_CACHE[key]


def _shard_inputs(x, qweight, qzeros, scales, bias):
    T, I = x.shape
    O = qweight.shape[0]
    o_pad = -(-O // (16 * NCORES)) * (16 * NCORES)
    o_sh = o_pad // NCORES
    KT = I // 128

    # x rows permuted to nibble k-tile order: row kt*128+p <- i=8*((kt%4)*128+p)+kt//4
    kt = np.arange(KT)
    p = np.arange(128)
    idx = (8 * ((kt % 4)[:, None] * 128 + p[None, :]) + (kt // 4)[:, None]).reshape(-1)
    xT = np.ascontiguousarray(np.asarray(x).T[idx])

    def pad_rows(a):
        if a.shape[0] == o_pad:
            return a
        pad = np.zeros((o_pad - a.shape[0],) + a.shape[1:], a.dtype)
        return np.concatenate([a, pad], axis=0)

    qw_p = pad_rows(np.asarray(qweight))
    qz_p = pad_rows(np.asarray(qzeros))
    sc_p = pad_rows(np.asarray(scales))
    b_p = pad_rows(np.asarray(bias))
    in_maps = []
    for c in range(NCORES):
        rows = slice(c * o_sh, (c + 1) * o_sh)
        # packed-weight transpose (layout only): qwT[p, c4, o] = qw[o, c4*128+p]
        qwT = np.ascontiguousarray(
            qw_p[rows].T.reshape(4, 128, o_sh).transpose(1, 0, 2)
        )
        scs = np.ascontiguousarray(sc_p[rows])
        in_maps.append(
            {
                "xT": xT,
                "qwT": qwT,
                "qz": np.ascontiguousarray(qz_p[rows]),
                "scT": np.ascontiguousarray(scs.T),
                "b": np.ascontiguousarray(b_p[rows]).reshape(1, o_sh),
            }
        )
    return in_maps, T, I, O, o_sh


def _run(x, qweight, qzeros, scales, bias, trace=False, **kw):
    in_maps, T, I, O, o_sh = _shard_inputs(x, qweight, qzeros, scales, bias)
    nc = _get_nc(T, I, o_sh)
    res = run_bass_kernel_spmd(nc, in_maps, list(range(NCORES)), trace=trace, **kw)
    out = np.concatenate([res.results[c]["out"] for c in range(NCORES)], axis=1)
    return out[:, :O], res


def kernel(x, qweight, qzeros, scales, bias):
    out, _ = _run(x, qweight, qzeros, scales, bias)
    return out
